# revision 5
# baseline (speedup 1.0000x reference)
"""Trainium2 Bass kernel for nn_BayesModel_75737453297749 (vq_codebook).

Math (per batch b):
    xc = W @ x_b                          # [256, 9216] projection (1x1 conv)
    mm_f = ||xc_p - s_fg||^2 per pixel p  # expanded: ||xc_p||^2 - 2 s.xc_p + ||s||^2
    ll_f = 2*sigmoid(-mm_f)   (the 2x cancels in the posterior)
    post_fg = p_fg*ll_f / (p_fg*ll_f + p_bg*ll_b)
    nt_f = sum_p post_fg ;  cs_f = xc @ post_fg
    n_new = a*n + nt ;  s_new = b*s + cs/nt

Device layout: xcT (pixels on partitions, channels on free dim), computed as
    xcT[p, c] = sum_cin x[cin, p] * WT[cin, c]
with the weight matrix augmented by two extra columns -2*W^T s_fg / -2*W^T s_bg
so the s.xc dot products fall out of the projection matmul for free.
The ||xc_p||^2 reduction is a fused ACT Square+accum. The backward
contractions over pixels (cs, nt) are 72 accumulating matmuls with the
[post_fg|post_bg] pair as stationary and [xc|ones] as moving operand.

Sharding: data-parallel over batch, 2 batches per core on 8 cores.
"""
import sys

sys.path.insert(0, "/opt/trn_rl_repo")

import numpy as np

import concourse.bacc as bacc
import concourse.bass as bass
import concourse.mybir as mybir
import concourse.tile as tile
from concourse import bass_utils

AF = mybir.ActivationFunctionType
ALU = mybir.AluOpType
F32 = mybir.dt.float32

ALPHA = 0.8
BETA = 0.8

B, CIN, H, W_ = 16, 1024, 96, 96
PX = H * W_            # 9216 pixels
CC = 256               # codebook channels
NCORES = 8
BPC = B // NCORES      # batches per core = 2
NT = PX // 128         # 72 pixel-tiles of 128
NG = 9                 # 9 groups of 1024 pixels
GPX = 1024             # pixels per x-chunk
KC = CIN // 128        # 8 contraction chunks
NW = 258               # wta cols: [-2WTs_fg | -2WTs_bg | WT(256)]
SLOT = 259             # buf slot: [aug(2) | xc(256) | ones(1)]

_cache = {}


def _build(rep=1):
    """rep>1 wraps the whole per-core computation in a hardware loop that
    repeats it rep times — used only for differential wall-clock timing
    (axon transfer/dispatch overhead cancels between two rep values)."""
    import contextlib

    nc = bacc.Bacc("TRN2", debug=False, num_devices=NCORES)

    x_d = nc.dram_tensor("x", [BPC, CIN, PX], F32, kind="ExternalInput")
    wta_d = nc.dram_tensor("wta", [128, BPC * KC * NW], F32, kind="ExternalInput")
    scal_d = nc.dram_tensor("scal", [128, 4 * BPC], F32, kind="ExternalInput")
    id_d = nc.dram_tensor("ident", [128, 128], F32, kind="ExternalInput")
    post_d = nc.dram_tensor("post", [BPC, NT, 128], F32, kind="ExternalOutput")
    stats_d = nc.dram_tensor("stats", [BPC, 2, 257], F32, kind="ExternalOutput")

    with tile.TileContext(nc) as tc:
        with (
            tc.tile_pool(name="singles", bufs=1) as singles,
            tc.tile_pool(name="xp", bufs=16) as xpool,
            tc.tile_pool(name="pip", bufs=2) as pip,
            tc.tile_pool(name="scr", bufs=3) as scr,
            tc.tile_pool(name="ob", bufs=2) as obp,
            tc.tile_pool(name="st", bufs=2) as stp,
            tc.tile_pool(name="ps", bufs=6, space="PSUM") as psp,
            tc.tile_pool(name="bw", bufs=1, space="PSUM") as bwp,
            tc.tile_pool(name="tp", bufs=1, space="PSUM") as tpp,
        ):
            wta = singles.tile([128, BPC * KC * NW], F32)
            nc.sync.dma_start(wta[:], wta_d[:])
            scal = singles.tile([128, 4 * BPC], F32)
            nc.sync.dma_start(scal[:], scal_d[:])
            ident = singles.tile([128, 128], F32)
            nc.sync.dma_start(ident[:], id_d[:])
            buf = singles.tile([128, SLOT * NT], F32)
            nc.vector.memset(buf[:, 258::SLOT], 1.0)  # ones col of every slot

            loop = (
                contextlib.nullcontext()
                if rep == 1
                else tc.For_i(0, rep, 1, hint_engines=(mybir.EngineType.PE,))
            )
            with loop:
              for b in range(BPC):
                sq_all = pip.tile([128, NT], F32, tag="sq")
                for ng in range(NG):
                    xts = []
                    for k in range(KC):
                        xt = xpool.tile([128, GPX], F32, tag="x")
                        nc.sync.dma_start(
                            xt[:],
                            x_d[b, 128 * k : 128 * (k + 1),
                                GPX * ng : GPX * (ng + 1)],
                        )
                        xts.append(xt)
                    for w2 in range(2):
                        for j in range(4):
                            t = ng * 8 + w2 * 4 + j
                            ps = psp.tile([128, NW], F32)
                            for k in range(KC):
                                nc.tensor.matmul(
                                    ps[:],
                                    xts[k][:, 512 * w2 + 128 * j : 512 * w2 + 128 * (j + 1)],
                                    wta[:, (b * KC + k) * NW : (b * KC + k + 1) * NW],
                                    start=(k == 0),
                                    stop=(k == KC - 1),
                                )
                            # ||xc_p||^2 via fused square+row-sum (ACT)
                            sc = scr.tile([128, CC], F32, tag="sc")
                            nc.scalar.activation(
                                sc[:], ps[:, 2:NW], AF.Square,
                                accum_out=sq_all[:, t : t + 1],
                            )
                            # keep [-2s.xc | xc] for later reuse (DVE)
                            nc.vector.tensor_copy(
                                buf[:, SLOT * t : SLOT * t + NW], ps[:, 0:NW]
                            )

                # ---- per-pixel posterior pipeline on [128, 72] tiles ----
                u_fg = pip.tile([128, NT], F32, tag="ufg")
                u_bg = pip.tile([128, NT], F32, tag="ubg")
                nc.vector.tensor_tensor(u_fg[:], sq_all[:], buf[:, 0::SLOT], op=ALU.add)
                nc.vector.tensor_tensor(u_bg[:], sq_all[:], buf[:, 1::SLOT], op=ALU.add)
                sig_fg = pip.tile([128, NT], F32, tag="sfg")
                sig_bg = pip.tile([128, NT], F32, tag="sbg")
                # sigmoid(-(u + ||s||^2)) ; bias = -||s||^2 comes in via scal
                nc.scalar.activation(
                    sig_fg[:], u_fg[:], AF.Sigmoid,
                    bias=scal[:, 4 * b : 4 * b + 1], scale=-1.0,
                )
                nc.scalar.activation(
                    sig_bg[:], u_bg[:], AF.Sigmoid,
                    bias=scal[:, 4 * b + 1 : 4 * b + 2], scale=-1.0,
                )
                num_fg = pip.tile([128, NT], F32, tag="nfg")
                num_bg = pip.tile([128, NT], F32, tag="nbg")
                nc.vector.tensor_scalar_mul(num_fg[:], sig_fg[:], scal[:, 4 * b + 2 : 4 * b + 3])
                nc.vector.tensor_scalar_mul(num_bg[:], sig_bg[:], scal[:, 4 * b + 3 : 4 * b + 4])
                den = pip.tile([128, NT], F32, tag="den")
                nc.vector.tensor_tensor(den[:], num_fg[:], num_bg[:], op=ALU.add)
                rec = pip.tile([128, NT], F32, tag="rec")
                nc.vector.reciprocal(rec[:], den[:])
                pf = pip.tile([128, NT], F32, tag="pf")
                nc.vector.tensor_tensor(pf[:], num_fg[:], rec[:], op=ALU.mult)
                postbuf = pip.tile([128, 2 * NT], F32, tag="postbuf")
                nc.vector.tensor_copy(postbuf[:, 0::2], pf[:])
                nc.vector.tensor_tensor(
                    postbuf[:, 1::2], num_bg[:], rec[:], op=ALU.mult
                )

                # ---- backward: cs/nt via 72 accumulating matmuls ----
                bw = bwp.tile([2, 257], F32)
                for t in range(NT):
                    nc.tensor.matmul(
                        bw[:],
                        postbuf[:, 2 * t : 2 * t + 2],
                        buf[:, SLOT * t + 2 : SLOT * (t + 1)],
                        start=(t == 0),
                        stop=(t == NT - 1),
                    )
                stat_sb = stp.tile([2, 257], F32)
                nc.vector.tensor_copy(stat_sb[:], bw[:])
                nc.sync.dma_start(stats_d[b], stat_sb[:])

                # ---- post_fg out: transpose [128,72] -> [72,128] on PE ----
                tp = tpp.tile([NT, 128], F32)
                nc.tensor.transpose(tp[:], pf[:], ident[:])
                ob = obp.tile([NT, 128], F32)
                nc.vector.tensor_copy(ob[:], tp[:])
                nc.sync.dma_start(post_d[b], ob[:])

    nc.compile()
    return nc


def _host_prep(x, W, n_fg, n_bg, s_fg, s_bg):
    """Build per-core input maps."""
    x = np.ascontiguousarray(x, dtype=np.float32)
    W = np.asarray(W, dtype=np.float32)
    WT = W.T  # [CIN, CC]
    aug_fg = -2.0 * (s_fg @ W)  # [B, CIN]
    aug_bg = -2.0 * (s_bg @ W)
    c_fg = -np.sum(s_fg * s_fg, axis=1)  # -||s||^2, [B]
    c_bg = -np.sum(s_bg * s_bg, axis=1)
    p_fg = n_fg / (n_fg + n_bg)
    p_bg = 1.0 - p_fg

    ident = np.eye(128, dtype=np.float32)
    in_maps = []
    for c in range(NCORES):
        bs = range(c * BPC, (c + 1) * BPC)
        wta = np.empty((128, BPC * KC, NW), np.float32)
        for bl, b in enumerate(bs):
            for k in range(KC):
                g = bl * KC + k
                rows = slice(128 * k, 128 * (k + 1))
                wta[:, g, 0] = aug_fg[b, rows]
                wta[:, g, 1] = aug_bg[b, rows]
                wta[:, g, 2:] = WT[rows, :]
        scal = np.empty((128, 4 * BPC), np.float32)
        for bl, b in enumerate(bs):
            scal[:, 4 * bl + 0] = c_fg[b]
            scal[:, 4 * bl + 1] = c_bg[b]
            scal[:, 4 * bl + 2] = p_fg[b]
            scal[:, 4 * bl + 3] = p_bg[b]
        in_maps.append({
            "x": x[c * BPC : (c + 1) * BPC].reshape(BPC, CIN, PX),
            "wta": np.ascontiguousarray(wta.reshape(128, BPC * KC * NW)),
            "scal": scal,
            "ident": ident,
        })
    return in_maps


def _finalize(results, n_fg, n_bg, s_fg, s_bg):
    post = np.concatenate([r["post"] for r in results], axis=0)  # [B, 72, 128]
    stats = np.concatenate([r["stats"] for r in results], axis=0)  # [B, 2, 257]
    post_fg = post.reshape(B, 1, H, W_)
    cs_fg, nt_fg = stats[:, 0, :CC], stats[:, 0, CC]
    cs_bg, nt_bg = stats[:, 1, :CC], stats[:, 1, CC]
    with np.errstate(invalid="ignore", divide="ignore"):
        n_fg_new = ALPHA * n_fg + nt_fg
        n_bg_new = ALPHA * n_bg + nt_bg
        s_fg_new = BETA * s_fg + cs_fg / nt_fg[:, None]
        s_bg_new = BETA * s_bg + cs_bg / nt_bg[:, None]
    return post_fg, n_fg_new, n_bg_new, s_fg_new, s_bg_new


def kernel(x, W, n_fg, n_bg, s_fg, s_bg):
    x = np.asarray(x, dtype=np.float32)
    W = np.asarray(W, dtype=np.float32)
    n_fg = np.asarray(n_fg, dtype=np.float32)
    n_bg = np.asarray(n_bg, dtype=np.float32)
    s_fg = np.asarray(s_fg, dtype=np.float32)
    s_bg = np.asarray(s_bg, dtype=np.float32)

    if "nc" not in _cache:
        _cache["nc"] = _build()
    nc = _cache["nc"]
    in_maps = _host_prep(x, W, n_fg, n_bg, s_fg, s_bg)
    res = bass_utils.run_bass_kernel_spmd(nc, in_maps, core_ids=list(range(NCORES)))
    return _finalize(res.results, n_fg, n_bg, s_fg, s_bg)


if __name__ == "__main__":
    import reference as R

    inputs = R.setup_inputs()
    out = kernel(**{k: np.asarray(v) for k, v in inputs.items()})
    for i, o in enumerate(out):
        print(i, o.shape, o.dtype, "nan%:", float(np.isnan(o).mean()))


# revision 18
# speedup vs baseline: 2.3779x; 2.3779x over previous
"""Trainium2 Bass kernel for nn_BayesModel_75737453297749 (vq_codebook).

Math (per batch b):
    xc = W @ x_b                          # [256, 9216] projection (1x1 conv)
    mm_f = ||xc_p - s_fg||^2 per pixel p  # expanded: ||xc_p||^2 - 2 s.xc_p + ||s||^2
    ll_f = 2*sigmoid(-mm_f)   (the 2x cancels in the posterior)
    post_fg = p_fg*ll_f / (p_fg*ll_f + p_bg*ll_b)
    nt_f = sum_p post_fg ;  cs_f = xc @ post_fg
    n_new = a*n + nt ;  s_new = b*s + cs/nt

Device layout: xcT (pixels on partitions, channels on free dim), computed as
    xcT[p, c] = sum_cin x[cin, p] * WT[cin, c]
with the weight matrix augmented by two extra columns -2*W^T s_fg / -2*W^T s_bg
so the s.xc dot products fall out of the projection matmul for free.
The ||xc_p||^2 reduction is a fused ACT Square+accum. The backward
contractions over pixels (cs, nt) are 72 accumulating matmuls with the
[post_fg|post_bg] pair as stationary and [xc|ones] as moving operand.

Sharding: data-parallel over batch, 2 batches per core on 8 cores.
"""
import sys

sys.path.insert(0, "/opt/trn_rl_repo")

import numpy as np

import concourse.bacc as bacc
import concourse.bass as bass
import concourse.mybir as mybir
import concourse.tile as tile
from concourse import bass_utils

AF = mybir.ActivationFunctionType
ALU = mybir.AluOpType
F32 = mybir.dt.float32
F32R = mybir.dt.float32r  # fp32 storage, 4x faster PE mode (N>=256)

ALPHA = 0.8
BETA = 0.8

B, CIN, H, W_ = 16, 1024, 96, 96
PX = H * W_            # 9216 pixels
CC = 256               # codebook channels
NCORES = 8
BPC = B // NCORES      # batches per core = 2
NT = PX // 128         # 72 pixel-tiles of 128
GRP = 768              # pixels per x-load group (one DMA loads all 8 k-chunks)
NGRP = PX // GRP       # 12 groups per batch
TPG = GRP // 128       # 6 pixel-tiles per group
KC = CIN // 128        # 8 contraction chunks
NW = 258               # wta cols: [-2WTs_fg | -2WTs_bg | WT(256)]
SLOT = 259             # buf slot: [aug(2) | xc(256) | ones(1)]

_cache = {}


def _build(rep=1):
    """rep>1 wraps the whole per-core computation in a hardware loop that
    repeats it rep times — used only for differential wall-clock timing
    (axon transfer/dispatch overhead cancels between two rep values)."""
    import contextlib

    nc = bacc.Bacc("TRN2", debug=False, num_devices=NCORES)

    x_d = nc.dram_tensor("x", [BPC, CIN, PX], F32R, kind="ExternalInput")
    wta_d = nc.dram_tensor("wta", [128, BPC * KC * NW], F32R, kind="ExternalInput")
    scal_d = nc.dram_tensor("scal", [128, 4 * BPC], F32, kind="ExternalInput")
    id_d = nc.dram_tensor("ident", [128, 128], F32, kind="ExternalInput")
    post_d = nc.dram_tensor("post", [BPC, NT, 128], F32, kind="ExternalOutput")
    stats_d = nc.dram_tensor("stats", [BPC, 2, 257], F32, kind="ExternalOutput")

    with tile.TileContext(nc) as tc:
        with (
            tc.tile_pool(name="singles", bufs=1) as singles,
            tc.tile_pool(name="xp", bufs=3) as xpool,
            tc.tile_pool(name="pip", bufs=2) as pip,
            tc.tile_pool(name="scr", bufs=3) as scr,
            tc.tile_pool(name="ob", bufs=2) as obp,
            tc.tile_pool(name="st", bufs=2) as stp,
            tc.tile_pool(name="ps", bufs=6, space="PSUM") as psp,
            tc.tile_pool(name="bw", bufs=1, space="PSUM") as bwp,
            tc.tile_pool(name="tp", bufs=1, space="PSUM") as tpp,
        ):
            wta = singles.tile([128, BPC * KC * NW], F32R)
            nc.sync.dma_start(wta[:], wta_d[:])
            scal = singles.tile([128, 4 * BPC], F32)
            nc.sync.dma_start(scal[:], scal_d[:])
            ident = singles.tile([128, 128], F32)
            nc.sync.dma_start(ident[:], id_d[:])
            buf = singles.tile([128, SLOT * NT], F32R)
            nc.vector.memset(buf[:, 258::SLOT].bitcast(F32), 1.0)  # ones cols

            loop = (
                contextlib.nullcontext()
                if rep == 1
                else tc.For_i(0, rep, 1, hint_engines=(mybir.EngineType.PE,))
            )
            with loop:
              for b in range(BPC):
                sq_all = pip.tile([128, NT], F32, tag="sq")
                xv = x_d[b].rearrange("(k p) c -> p k c", p=128)  # [128, 8, PX]
                for g in range(NGRP):
                    # one DMA: all 8 k-chunks of this pixel group (3 MB)
                    xg = xpool.tile([128, KC, GRP], F32R, tag="x")
                    nc.sync.dma_start(xg[:], xv[:, :, GRP * g : GRP * (g + 1)])
                    for j in range(TPG):
                        t = g * TPG + j
                        ps = psp.tile([128, NW], F32)
                        for k in range(KC):
                            nc.tensor.matmul(
                                ps[:],
                                xg[:, k, 128 * j : 128 * (j + 1)],
                                wta[:, (b * KC + k) * NW : (b * KC + k + 1) * NW],
                                start=(k == 0),
                                stop=(k == KC - 1),
                            )
                        # ||xc_p||^2 via fused square+row-sum (ACT)
                        sc = scr.tile([128, CC], F32, tag="sc")
                        nc.scalar.activation(
                            sc[:], ps[:, 2:NW], AF.Square,
                            accum_out=sq_all[:, t : t + 1],
                        )
                        # keep [-2s.xc | xc] for later reuse (DVE)
                        nc.vector.tensor_copy(
                            buf[:, SLOT * t : SLOT * t + NW], ps[:, 0:NW]
                        )

                # ---- per-pixel posterior pipeline on [128, 72] tiles ----
                u_fg = pip.tile([128, NT], F32, tag="ufg")
                u_bg = pip.tile([128, NT], F32, tag="ubg")
                nc.vector.tensor_tensor(u_fg[:], sq_all[:], buf[:, 0::SLOT].bitcast(F32), op=ALU.add)
                nc.vector.tensor_tensor(u_bg[:], sq_all[:], buf[:, 1::SLOT].bitcast(F32), op=ALU.add)
                sig_fg = pip.tile([128, NT], F32, tag="sfg")
                sig_bg = pip.tile([128, NT], F32, tag="sbg")
                # sigmoid(-(u + ||s||^2)) ; bias = -||s||^2 comes in via scal
                nc.scalar.activation(
                    sig_fg[:], u_fg[:], AF.Sigmoid,
                    bias=scal[:, 4 * b : 4 * b + 1], scale=-1.0,
                )
                nc.scalar.activation(
                    sig_bg[:], u_bg[:], AF.Sigmoid,
                    bias=scal[:, 4 * b + 1 : 4 * b + 2], scale=-1.0,
                )
                num_fg = pip.tile([128, NT], F32, tag="nfg")
                num_bg = pip.tile([128, NT], F32, tag="nbg")
                nc.vector.tensor_scalar_mul(num_fg[:], sig_fg[:], scal[:, 4 * b + 2 : 4 * b + 3])
                nc.vector.tensor_scalar_mul(num_bg[:], sig_bg[:], scal[:, 4 * b + 3 : 4 * b + 4])
                den = pip.tile([128, NT], F32, tag="den")
                nc.vector.tensor_tensor(den[:], num_fg[:], num_bg[:], op=ALU.add)
                rec = pip.tile([128, NT], F32, tag="rec")
                nc.vector.reciprocal(rec[:], den[:])
                pf = pip.tile([128, NT], F32, tag="pf")
                nc.vector.tensor_tensor(pf[:], num_fg[:], rec[:], op=ALU.mult)
                postbuf = pip.tile([128, 2 * NT], F32, tag="postbuf")
                nc.vector.tensor_copy(postbuf[:, 0::2], pf[:])
                nc.vector.tensor_tensor(
                    postbuf[:, 1::2], num_bg[:], rec[:], op=ALU.mult
                )

                # ---- backward: cs/nt via 72 accumulating matmuls ----
                bw = bwp.tile([2, 257], F32)
                for t in range(NT):
                    nc.tensor.matmul(
                        bw[:],
                        postbuf[:, 2 * t : 2 * t + 2],
                        buf[:, SLOT * t + 2 : SLOT * (t + 1)].bitcast(F32),
                        start=(t == 0),
                        stop=(t == NT - 1),
                    )
                # stores go on ACT's HWDGE queue so they never head-of-line
                # block the x prefetch stream on the sync queue
                stat_sb = stp.tile([2, 257], F32)
                nc.vector.tensor_copy(stat_sb[:], bw[:])
                nc.scalar.dma_start(stats_d[b], stat_sb[:])

                # ---- post_fg out: transpose [128,72] -> [72,128] on PE ----
                tp = tpp.tile([NT, 128], F32)
                nc.tensor.transpose(tp[:], pf[:], ident[:])
                ob = obp.tile([NT, 128], F32)
                nc.vector.tensor_copy(ob[:], tp[:])
                nc.scalar.dma_start(post_d[b], ob[:])

    nc.compile()
    return nc


def _host_prep(x, W, n_fg, n_bg, s_fg, s_bg):
    """Build per-core input maps."""
    x = np.ascontiguousarray(x, dtype=np.float32)
    W = np.asarray(W, dtype=np.float32)
    WT = W.T  # [CIN, CC]
    aug_fg = -2.0 * (s_fg @ W)  # [B, CIN]
    aug_bg = -2.0 * (s_bg @ W)
    c_fg = -np.sum(s_fg * s_fg, axis=1)  # -||s||^2, [B]
    c_bg = -np.sum(s_bg * s_bg, axis=1)
    p_fg = n_fg / (n_fg + n_bg)
    p_bg = 1.0 - p_fg

    ident = np.eye(128, dtype=np.float32)
    in_maps = []
    for c in range(NCORES):
        bs = range(c * BPC, (c + 1) * BPC)
        wta = np.empty((128, BPC * KC, NW), np.float32)
        for bl, b in enumerate(bs):
            for k in range(KC):
                g = bl * KC + k
                rows = slice(128 * k, 128 * (k + 1))
                wta[:, g, 0] = aug_fg[b, rows]
                wta[:, g, 1] = aug_bg[b, rows]
                wta[:, g, 2:] = WT[rows, :]
        scal = np.empty((128, 4 * BPC), np.float32)
        for bl, b in enumerate(bs):
            scal[:, 4 * bl + 0] = c_fg[b]
            scal[:, 4 * bl + 1] = c_bg[b]
            scal[:, 4 * bl + 2] = p_fg[b]
            scal[:, 4 * bl + 3] = p_bg[b]
        in_maps.append({
            "x": x[c * BPC : (c + 1) * BPC].reshape(BPC, CIN, PX),
            "wta": np.ascontiguousarray(wta.reshape(128, BPC * KC * NW)),
            "scal": scal,
            "ident": ident,
        })
    return in_maps


def _finalize(results, n_fg, n_bg, s_fg, s_bg):
    post = np.concatenate([r["post"] for r in results], axis=0)  # [B, 72, 128]
    stats = np.concatenate([r["stats"] for r in results], axis=0)  # [B, 2, 257]
    post_fg = post.reshape(B, 1, H, W_)
    cs_fg, nt_fg = stats[:, 0, :CC], stats[:, 0, CC]
    cs_bg, nt_bg = stats[:, 1, :CC], stats[:, 1, CC]
    with np.errstate(invalid="ignore", divide="ignore"):
        n_fg_new = ALPHA * n_fg + nt_fg
        n_bg_new = ALPHA * n_bg + nt_bg
        s_fg_new = BETA * s_fg + cs_fg / nt_fg[:, None]
        s_bg_new = BETA * s_bg + cs_bg / nt_bg[:, None]
    return post_fg, n_fg_new, n_bg_new, s_fg_new, s_bg_new


def kernel(x, W, n_fg, n_bg, s_fg, s_bg):
    x = np.asarray(x, dtype=np.float32)
    W = np.asarray(W, dtype=np.float32)
    n_fg = np.asarray(n_fg, dtype=np.float32)
    n_bg = np.asarray(n_bg, dtype=np.float32)
    s_fg = np.asarray(s_fg, dtype=np.float32)
    s_bg = np.asarray(s_bg, dtype=np.float32)

    if "nc" not in _cache:
        _cache["nc"] = _build()
    nc = _cache["nc"]
    in_maps = _host_prep(x, W, n_fg, n_bg, s_fg, s_bg)
    res = bass_utils.run_bass_kernel_spmd(nc, in_maps, core_ids=list(range(NCORES)))
    return _finalize(res.results, n_fg, n_bg, s_fg, s_bg)


if __name__ == "__main__":
    import reference as R

    inputs = R.setup_inputs()
    out = kernel(**{k: np.asarray(v) for k, v in inputs.items()})
    for i, o in enumerate(out):
        print(i, o.shape, o.dtype, "nan%:", float(np.isnan(o).mean()))


# revision 23
# speedup vs baseline: 8.0986x; 3.4058x over previous
"""Trainium2 Bass kernel for nn_BayesModel_75737453297749 (vq_codebook).

Math (per batch b):
    xc = W @ x_b                          # [256, 9216] projection (1x1 conv)
    mm_f = ||xc_p - s_fg||^2 per pixel p  # expanded: ||xc_p||^2 - 2 s.xc_p + ||s||^2
    ll_f = 2*sigmoid(-mm_f)   (the 2x cancels in the posterior)
    post_fg = p_fg*ll_f / (p_fg*ll_f + p_bg*ll_b)
    nt_f = sum_p post_fg ;  cs_f = xc @ post_fg
    n_new = a*n + nt ;  s_new = b*s + cs/nt

Device layout: xcT (pixels on partitions, channels on free dim), computed as
    xcT[p, c] = sum_cin x[cin, p] * WT[cin, c]
with the weight matrix augmented by two extra columns -2*W^T s_fg / -2*W^T s_bg
so the s.xc dot products fall out of the projection matmul for free.
The ||xc_p||^2 reduction is a fused ACT Square+accum. The backward
contractions over pixels (cs, nt) are 72 accumulating matmuls with the
[post_fg|post_bg] pair as stationary and [xc|ones] as moving operand.

Sharding: data-parallel over batch, 2 batches per core on 8 cores.
"""
import sys

sys.path.insert(0, "/opt/trn_rl_repo")

import numpy as np

import concourse.bacc as bacc
import concourse.bass as bass
import concourse.mybir as mybir
import concourse.tile as tile
from concourse import bass_utils

AF = mybir.ActivationFunctionType
ALU = mybir.AluOpType
F32 = mybir.dt.float32
F32R = mybir.dt.float32r  # fp32 storage, 4x faster PE mode (N>=256)

ALPHA = 0.8
BETA = 0.8

B, CIN, H, W_ = 16, 1024, 96, 96
PX = H * W_            # 9216 pixels
CC = 256               # codebook channels
NCORES = 8
BPC = B // NCORES      # batches per core = 2
NT = PX // 128         # 72 pixel-tiles of 128
GRP = 768              # pixels per x-load group (one DMA loads all 8 k-chunks)
NGRP = PX // GRP       # 12 groups per batch
TPG = GRP // 128       # 6 pixel-tiles per group
KC = CIN // 128        # 8 contraction chunks
NW = 258               # wta cols: [-2WTs_fg | -2WTs_bg | WT(256)]
SLOT = 260             # buf slot: [aug(2) | xc(256) | ones(1) | zero(1)]

_cache = {}


def _build(rep=1, dma_split=1):
    """rep>1 wraps the whole per-core computation in a hardware loop that
    repeats it rep times — used only for differential wall-clock timing
    (axon transfer/dispatch overhead cancels between two rep values).
    dma_split: number of HWDGE queues to spread x-load DMAs across."""
    import contextlib

    nc = bacc.Bacc("TRN2", debug=False, num_devices=NCORES)

    x_d = nc.dram_tensor("x", [BPC, CIN, PX], F32R, kind="ExternalInput")
    wta_d = nc.dram_tensor("wta", [128, BPC * KC * NW], F32R, kind="ExternalInput")
    scal_d = nc.dram_tensor("scal", [128, 4 * BPC], F32, kind="ExternalInput")
    id_d = nc.dram_tensor("ident", [128, 128], F32, kind="ExternalInput")
    post_d = nc.dram_tensor("post", [BPC, NT, 128], F32, kind="ExternalOutput")
    stats_d = nc.dram_tensor("stats", [BPC, 2, 257], F32, kind="ExternalOutput")

    with tile.TileContext(nc) as tc:
        with (
            tc.tile_pool(name="singles", bufs=1) as singles,
            tc.tile_pool(name="xp", bufs=3) as xpool,
            tc.tile_pool(name="pip", bufs=2) as pip,
            tc.tile_pool(name="scr", bufs=3) as scr,
            tc.tile_pool(name="ob", bufs=2) as obp,
            tc.tile_pool(name="st", bufs=2) as stp,
            tc.tile_pool(name="ps", bufs=6, space="PSUM") as psp,
            tc.tile_pool(name="bw", bufs=1, space="PSUM") as bwp,
            tc.tile_pool(name="tp", bufs=1, space="PSUM") as tpp,
        ):
            wta = singles.tile([128, BPC * KC * NW], F32R)
            nc.sync.dma_start(wta[:], wta_d[:])
            scal = singles.tile([128, 4 * BPC], F32)
            nc.sync.dma_start(scal[:], scal_d[:])
            ident = singles.tile([128, 128], F32)
            nc.sync.dma_start(ident[:], id_d[:])
            buf = singles.tile([128, SLOT * NT], F32R)
            nc.vector.memset(buf[:].bitcast(F32), 0.0)   # zero pad cols
            nc.vector.memset(buf[:, 258::SLOT].bitcast(F32), 1.0)  # ones cols
            postbuf = singles.tile([128, 2 * NT], F32R)
            nc.vector.memset(postbuf[:].bitcast(F32), 0.0)

            loop = (
                contextlib.nullcontext()
                if rep == 1
                else tc.For_i(0, rep, 1, hint_engines=(mybir.EngineType.PE,))
            )
            with loop:
              for b in range(BPC):
                sq_all = pip.tile([128, NT], F32, tag="sq")
                xv = x_d[b].rearrange("(k p) c -> p k c", p=128)  # [128, 8, PX]
                for g in range(NGRP):
                    # one DMA: all 8 k-chunks of this pixel group (3 MB)
                    xg = xpool.tile([128, KC, GRP], F32R, tag="x")
                    eng = [nc.sync, nc.scalar, nc.gpsimd][(b * NGRP + g) % dma_split]
                    eng.dma_start(xg[:], xv[:, :, GRP * g : GRP * (g + 1)])
                    for j in range(TPG):
                        t = g * TPG + j
                        ps = psp.tile([128, NW], F32)
                        for k in range(KC):
                            nc.tensor.matmul(
                                ps[:],
                                xg[:, k, 128 * j : 128 * (j + 1)],
                                wta[:, (b * KC + k) * NW : (b * KC + k + 1) * NW],
                                start=(k == 0),
                                stop=(k == KC - 1),
                            )
                        # ||xc_p||^2 via fused square+row-sum (ACT)
                        sc = scr.tile([128, CC], F32, tag="sc")
                        nc.scalar.activation(
                            sc[:], ps[:, 2:NW], AF.Square,
                            accum_out=sq_all[:, t : t + 1],
                        )
                        # keep [-2s.xc | xc] for later reuse (DVE)
                        nc.vector.tensor_copy(
                            buf[:, SLOT * t : SLOT * t + NW], ps[:, 0:NW]
                        )

                # ---- per-pixel posterior pipeline on [128, 72] tiles ----
                u_fg = pip.tile([128, NT], F32, tag="ufg")
                u_bg = pip.tile([128, NT], F32, tag="ubg")
                nc.vector.tensor_tensor(u_fg[:], sq_all[:], buf[:, 0::SLOT].bitcast(F32), op=ALU.add)
                nc.vector.tensor_tensor(u_bg[:], sq_all[:], buf[:, 1::SLOT].bitcast(F32), op=ALU.add)
                sig_fg = pip.tile([128, NT], F32, tag="sfg")
                sig_bg = pip.tile([128, NT], F32, tag="sbg")
                # sigmoid(-(u + ||s||^2)) ; bias = -||s||^2 comes in via scal
                nc.scalar.activation(
                    sig_fg[:], u_fg[:], AF.Sigmoid,
                    bias=scal[:, 4 * b : 4 * b + 1], scale=-1.0,
                )
                nc.scalar.activation(
                    sig_bg[:], u_bg[:], AF.Sigmoid,
                    bias=scal[:, 4 * b + 1 : 4 * b + 2], scale=-1.0,
                )
                num_fg = pip.tile([128, NT], F32, tag="nfg")
                num_bg = pip.tile([128, NT], F32, tag="nbg")
                nc.vector.tensor_scalar_mul(num_fg[:], sig_fg[:], scal[:, 4 * b + 2 : 4 * b + 3])
                nc.vector.tensor_scalar_mul(num_bg[:], sig_bg[:], scal[:, 4 * b + 3 : 4 * b + 4])
                den = pip.tile([128, NT], F32, tag="den")
                nc.vector.tensor_tensor(den[:], num_fg[:], num_bg[:], op=ALU.add)
                rec = pip.tile([128, NT], F32, tag="rec")
                nc.vector.reciprocal(rec[:], den[:])
                pf = pip.tile([128, NT], F32, tag="pf")
                nc.vector.tensor_tensor(pf[:], num_fg[:], rec[:], op=ALU.mult)
                nc.vector.tensor_copy(postbuf[:, 0 : 2 * NT : 2], pf[:])
                nc.vector.tensor_tensor(
                    postbuf[:, 1 : 2 * NT : 2], num_bg[:], rec[:], op=ALU.mult
                )

                # ---- backward: cs/nt via 72 accumulating matmuls ----
                bw = bwp.tile([2, 258], F32)
                for t in range(NT):
                    nc.tensor.matmul(
                        bw[:],
                        postbuf[:, 2 * t : 2 * t + 2],
                        buf[:, SLOT * t + 2 : SLOT * (t + 1)],
                        start=(t == 0),
                        stop=(t == NT - 1),
                    )
                # stores go on ACT's HWDGE queue so they never head-of-line
                # block the x prefetch stream on the sync queue
                stat_sb = stp.tile([2, 257], F32)
                nc.vector.tensor_copy(stat_sb[:], bw[:, 0:257])
                nc.scalar.dma_start(stats_d[b], stat_sb[:])

                # ---- post_fg out: transpose [128,72] -> [72,128] on PE ----
                tp = tpp.tile([NT, 128], F32)
                nc.tensor.transpose(tp[:], pf[:], ident[:])
                ob = obp.tile([NT, 128], F32)
                nc.vector.tensor_copy(ob[:], tp[:])
                nc.scalar.dma_start(post_d[b], ob[:])

    nc.compile()
    return nc


def _host_prep(x, W, n_fg, n_bg, s_fg, s_bg):
    """Build per-core input maps."""
    x = np.ascontiguousarray(x, dtype=np.float32)
    W = np.asarray(W, dtype=np.float32)
    WT = W.T  # [CIN, CC]
    aug_fg = -2.0 * (s_fg @ W)  # [B, CIN]
    aug_bg = -2.0 * (s_bg @ W)
    c_fg = -np.sum(s_fg * s_fg, axis=1)  # -||s||^2, [B]
    c_bg = -np.sum(s_bg * s_bg, axis=1)
    p_fg = n_fg / (n_fg + n_bg)
    p_bg = 1.0 - p_fg

    ident = np.eye(128, dtype=np.float32)
    in_maps = []
    for c in range(NCORES):
        bs = range(c * BPC, (c + 1) * BPC)
        wta = np.empty((128, BPC * KC, NW), np.float32)
        for bl, b in enumerate(bs):
            for k in range(KC):
                g = bl * KC + k
                rows = slice(128 * k, 128 * (k + 1))
                wta[:, g, 0] = aug_fg[b, rows]
                wta[:, g, 1] = aug_bg[b, rows]
                wta[:, g, 2:] = WT[rows, :]
        scal = np.empty((128, 4 * BPC), np.float32)
        for bl, b in enumerate(bs):
            scal[:, 4 * bl + 0] = c_fg[b]
            scal[:, 4 * bl + 1] = c_bg[b]
            scal[:, 4 * bl + 2] = p_fg[b]
            scal[:, 4 * bl + 3] = p_bg[b]
        in_maps.append({
            "x": x[c * BPC : (c + 1) * BPC].reshape(BPC, CIN, PX),
            "wta": np.ascontiguousarray(wta.reshape(128, BPC * KC * NW)),
            "scal": scal,
            "ident": ident,
        })
    return in_maps


def _finalize(results, n_fg, n_bg, s_fg, s_bg):
    post = np.concatenate([r["post"] for r in results], axis=0)  # [B, 72, 128]
    stats = np.concatenate([r["stats"] for r in results], axis=0)  # [B, 2, 257]
    post_fg = post.reshape(B, 1, H, W_)
    cs_fg, nt_fg = stats[:, 0, :CC], stats[:, 0, CC]
    cs_bg, nt_bg = stats[:, 1, :CC], stats[:, 1, CC]
    with np.errstate(invalid="ignore", divide="ignore"):
        n_fg_new = ALPHA * n_fg + nt_fg
        n_bg_new = ALPHA * n_bg + nt_bg
        s_fg_new = BETA * s_fg + cs_fg / nt_fg[:, None]
        s_bg_new = BETA * s_bg + cs_bg / nt_bg[:, None]
    return post_fg, n_fg_new, n_bg_new, s_fg_new, s_bg_new


def kernel(x, W, n_fg, n_bg, s_fg, s_bg):
    x = np.asarray(x, dtype=np.float32)
    W = np.asarray(W, dtype=np.float32)
    n_fg = np.asarray(n_fg, dtype=np.float32)
    n_bg = np.asarray(n_bg, dtype=np.float32)
    s_fg = np.asarray(s_fg, dtype=np.float32)
    s_bg = np.asarray(s_bg, dtype=np.float32)

    if "nc" not in _cache:
        _cache["nc"] = _build()
    nc = _cache["nc"]
    in_maps = _host_prep(x, W, n_fg, n_bg, s_fg, s_bg)
    res = bass_utils.run_bass_kernel_spmd(nc, in_maps, core_ids=list(range(NCORES)))
    return _finalize(res.results, n_fg, n_bg, s_fg, s_bg)


if __name__ == "__main__":
    import reference as R

    inputs = R.setup_inputs()
    out = kernel(**{k: np.asarray(v) for k, v in inputs.items()})
    for i, o in enumerate(out):
        print(i, o.shape, o.dtype, "nan%:", float(np.isnan(o).mean()))
